# revision 7
# baseline (speedup 1.0000x reference)
"""Causal multi-head attention on 8 Trainium2 NeuronCores (bf16/fp16).

Sharding: core c -> (batch b = c//2, head-group g = c%2 of 6 heads).
Host sums the two half-head partial outputs per batch.

v5 design (all-16-bit matmuls; exp as a 2-op fp16 bit trick; deep
software pipeline so the PE never waits on the exp chain):
  - projections: plain bf16 matmuls (K=128 x 6 k-tiles)
  - scores: bf16 K=64 matmuls; the two heads of a j-pair run in
    concurrent PE row groups (0,0)/(64,0) -> ~2x, writing one
    [128, 1024] staging tile ([head A | head B]); causal trim per tile
  - exp: 2-op Schraudolph on fp16 bits:
      pass1: y = fp16(S*k1 + c1)  (k1=32*log2e/8, c1=15*32+1024) so
             round-to-int is free in fp16's [1024,2048) window
      pass2: int16 (bits(y) - 25600) * 32 = fp16 bits of 2^t with a
             linear 5-bit mantissa; boundary tiles multiply by a
             {32|0} triangle mask instead (exact zeros)
    split between ACT (float path) and DVE (int16 2x) by booked cost
  - pipeline ring: scores(t) | pass1(t-1) | pass2(t-2) | PV(t-3), so PV
    never stalls the in-order PE queue behind the exp chain
  - PV: fp16 e x fp16 V_aug [128, 65] (ones col -> denominator row 64)
  - normalize: DVE reciprocal straight off the PSUM denom rows; the
    recip rows are partition-broadcast by doubling SBUF DMAs (no PE
    matmul on the recip path); TT mul -> zh bf16
  - W_O: plain bf16, f32 out via ACT/DVE copy + DMA
"""

import numpy as np

B = 4
S = 2048
D = 768
NH = 12
DH = 64
G = 2            # head groups (tensor parallel)
HPG = NH // G    # heads per group = 6
NP = HPG // 2    # head pairs per group = 3
ST = S // 128    # 16 s-tiles
QC = S // 512    # 4 q-chunks
N_CORES = 8
VS = 66          # per-(head,tile) stride in v_big (64 V + 1 ones + 1 pad)

FS = 32.0                                  # exponent fraction scale
K1 = FS * 1.4426950408889634 / 8.0         # pass1 scale
C1 = 15.0 * FS + 1024.0                    # pass1 bias
M32 = 32                                   # pass2 multiplier


def _split_drain_waits(nc, mybir, max_waits=1):
    """Walrus accepts one sync wait per instruction; hoist extras onto
    NoOps on the same engine (program order keeps the gating)."""
    for f in nc.m.functions:
        for bb in f.blocks:
            newlist = []
            for ins in bb.instructions:
                si = ins.sync_info
                if si is not None and si.on_wait and len(si.on_wait) > max_waits:
                    waits = list(si.on_wait)
                    for i, w in enumerate(waits[:-max_waits]):
                        d = mybir.InstNoOp(name=f"{ins.name}-sw{i}", ins=[], outs=[])
                        d.engine = ins.engine
                        d.sync_info = mybir.SyncInfo(on_wait=[w], on_update=[])
                        newlist.append(d)
                    ins.sync_info = mybir.SyncInfo(
                        on_wait=list(waits[-max_waits:]), on_update=list(si.on_update)
                    )
                newlist.append(ins)
            try:
                bb.instructions = newlist
            except Exception:
                bb.instructions.clear()
                bb.instructions.extend(newlist)


def build_program():
    import concourse.bass as bass
    import concourse.mybir as mybir
    import concourse.tile as tile
    from contextlib import ExitStack

    f32 = mybir.dt.float32
    bf16 = mybir.dt.bfloat16
    f16 = mybir.dt.float16
    i16 = mybir.dt.int16
    MULT = mybir.AluOpType.mult
    SUB = mybir.AluOpType.subtract
    ADD = mybir.AluOpType.add
    COPY = mybir.ActivationFunctionType.Copy
    IDENT = mybir.ActivationFunctionType.Identity

    nc = bass.Bass("TRN2")
    xT = nc.dram_tensor("xT", [D, S], bf16, kind="ExternalInput")
    wq = nc.dram_tensor("wq", [128, 6 * 384], bf16, kind="ExternalInput")
    wk = nc.dram_tensor("wk", [128, 6 * 384], bf16, kind="ExternalInput")
    wv = nc.dram_tensor("wv", [128, 6 * 384], bf16, kind="ExternalInput")
    wo = nc.dram_tensor("wo", [128, 3 * 768], bf16, kind="ExternalInput")
    bqd = nc.dram_tensor("bqd", [128, 3], f32, kind="ExternalInput")
    bkd = nc.dram_tensor("bkd", [128, 3], f32, kind="ExternalInput")
    mskd = nc.dram_tensor("mskd", [128, 1024], i16, kind="ExternalInput")
    ones2_d = nc.dram_tensor("ones2_d", [128, 128], f32, kind="ExternalInput")
    out = nc.dram_tensor("out", [S, D], f32, kind="ExternalOutput")

    ew_load = {"act": 0.0, "dve": 0.0}

    def ew_pick(act_ns, dve_ns):
        ca = ew_load["act"] + act_ns + 180
        cd = ew_load["dve"] + dve_ns + 180
        if ca <= cd:
            ew_load["act"] = ca
            return "act"
        ew_load["dve"] = cd
        return "dve"

    def ew_book(which, ns):
        ew_load[which] += ns

    with tile.TileContext(nc) as tc:
        with ExitStack() as _ctx:
            _e = _ctx.enter_context
            _e(nc.allow_low_precision(reason="bf16/fp16 attention pipeline"))
            wpool = _e(tc.tile_pool(name="weights", bufs=1))
            xpool = _e(tc.tile_pool(name="xt", bufs=6))
            qkpool = _e(tc.tile_pool(name="qk", bufs=2 * NP * QC))
            vpool = _e(tc.tile_pool(name="v", bufs=1))
            ypool = _e(tc.tile_pool(name="y16", bufs=2))
            epool = _e(tc.tile_pool(name="e", bufs=3))
            zhpool = _e(tc.tile_pool(name="zh", bufs=2))
            rrpool = _e(tc.tile_pool(name="rr", bufs=2))
            bcpool = _e(tc.tile_pool(name="bcs", bufs=2))
            opool = _e(tc.tile_pool(name="osb", bufs=3))
            stpool = _e(tc.tile_pool(name="st", bufs=2, space="PSUM"))
            pzpool = _e(tc.tile_pool(name="pz", bufs=3, space="PSUM"))
            mspool = _e(tc.tile_pool(name="ms", bufs=1, space="PSUM"))

            # ---- small constants ----
            bq_sb = wpool.tile([128, NP], f32, tag="bq")
            nc.sync.dma_start(bq_sb[:], bqd[:])
            bk_sb = wpool.tile([128, NP], f32, tag="bk")
            nc.sync.dma_start(bk_sb[:], bkd[:])
            msk = wpool.tile([128, 1024], i16, tag="msk")
            nc.gpsimd.dma_start(msk[:], mskd[:])
            ones2 = wpool.tile([128, 128], f32, tag="ones2")
            nc.gpsimd.dma_start(ones2[:], ones2_d[:])

            # ---- PE warmup (p-state) while input DMAs land ----
            wu = mspool.tile([128, 512], f32, tag="ms", name="wu")
            for _ in range(20):
                nc.tensor.matmul(wu[:, 0:128], ones2[:], ones2[:],
                                 start=True, stop=True)

            # ---- bulk inputs over two DMA queues ----
            xt = [xpool.tile([128, S], bf16, tag="xt", name=f"xt{a}")
                  for a in range(6)]
            for a in range(6):
                half = S // 2
                eng = nc.sync if a % 2 == 0 else nc.gpsimd
                eng.dma_start(xt[a][:, 0:half], xT[a * 128:(a + 1) * 128, 0:half])
                eng2 = nc.gpsimd if a % 2 == 0 else nc.sync
                eng2.dma_start(xt[a][:, half:S], xT[a * 128:(a + 1) * 128, half:S])
            wq_sb = wpool.tile([128, 6 * 384], bf16, tag="wq")
            nc.sync.dma_start(wq_sb[:], wq[:])
            wk_sb = wpool.tile([128, 6 * 384], bf16, tag="wk")
            nc.gpsimd.dma_start(wk_sb[:], wk[:])
            wv_sb = wpool.tile([128, 6 * 384], bf16, tag="wv")
            nc.sync.dma_start(wv_sb[:], wv[:])
            wo_sb = wpool.tile([128, 3 * 768], bf16, tag="wo")
            nc.gpsimd.dma_start(wo_sb[:], wo[:])

            # ---- persistent activation tiles ----
            qt = [[qkpool.tile([128, 512], bf16, tag="qk", name=f"qt{j}_{c}")
                   for c in range(QC)] for j in range(NP)]
            kt = [[qkpool.tile([128, 512], bf16, tag="qk", name=f"kt{j}_{c}")
                   for c in range(QC)] for j in range(NP)]
            v_big = vpool.tile([128, HPG * 16 * VS], f16, tag="v")
            vb = v_big[:].rearrange("p (h t m) -> p h t m", h=HPG, t=16)
            nc.gpsimd.memset(vb[:, :, :, 64:65], 1.0)

            zh_cs = [zhpool.tile([128, 4 * NP * 128], bf16, tag="zh",
                                 name=f"zh{c}") for c in range(QC)]

            def emit_proj_qk(c, j, which, pool):
                w_sb, b_sb, dst = ((wq_sb, bq_sb, qt[j][c]) if which == 0
                                   else (wk_sb, bk_sb, kt[j][c]))
                ps = pool.tile([128, 512], f32,
                               tag="st" if pool is stpool else "ms",
                               name="psqk")
                for a in range(6):
                    nc.tensor.matmul(
                        ps[:], w_sb[:, a * 384 + j * 128:a * 384 + (j + 1) * 128],
                        xt[a][:, c * 512:(c + 1) * 512],
                        start=(a == 0), stop=(a == 5))
                eng = ew_pick(512 * 0.833, 512 * 1.04)
                if eng == "act":
                    nc.scalar.activation(dst[:], ps[:], IDENT,
                                         bias=b_sb[:, j:j + 1], scale=1.0)
                else:
                    nc.vector.tensor_scalar_add(dst[:], ps[:], b_sb[:, j:j + 1])

            def emit_proj_v(st, pool):
                ps = pool.tile([128, 512], f32,
                               tag="st" if pool is stpool else "ms",
                               name="psv")
                for a in range(6):
                    nc.tensor.matmul(
                        ps[:, 0:384], xt[a][:, st * 128:(st + 1) * 128],
                        wv_sb[:, a * 384:(a + 1) * 384],
                        start=(a == 0), stop=(a == 5))
                dst = vb[:, :, st, 0:64]
                src = ps[:, 0:384].rearrange("p (h d) -> p h d", h=HPG)
                eng = ew_pick(384 * 0.833, 384 * 1.04)
                if eng == "act":
                    nc.scalar.copy(dst, src)
                else:
                    nc.vector.tensor_copy(dst, src)

            def proj_units(c, pool):
                if c >= QC:
                    return
                for j in range(NP):
                    yield lambda j=j: emit_proj_qk(c, j, 0, pool)
                    yield lambda j=j: emit_proj_qk(c, j, 1, pool)
                for st in range(4 * c, 4 * c + 4):
                    yield lambda st=st: emit_proj_v(st, pool)

            # prologue: chunk-0 projections through the (idle) stg pool
            for u in proj_units(0, stpool):
                u()

            # ---------- global software-pipelined attention ----------
            # unit = one k-tile of one (c, j). Stages within a unit step:
            # scores(u) | pass1(u-1) | pass2(u-2) | PV(u-3) | slot
            units = []      # list of dicts with stage closures
            post = {}       # unit index -> list of thunks (after PV stage)

            for c in range(QC):
                nt = 4 * c + 4
                for j in range(NP):
                    pz = [pzpool.tile([65, 512], f32, tag="pz",
                                      name=f"pz{c}_{j}_{hh}") for hh in range(2)]
                    jstate = {}

                    for t in range(nt):
                        r = t - 4 * c
                        qoff = max(r, 0) * 128
                        w = 512 - qoff
                        kc, ko = t // 4, (t % 4) * 128

                        def mk(c=c, j=j, t=t, r=r, qoff=qoff, w=w, kc=kc,
                               ko=ko, pz=pz, jstate=jstate, nt=nt):
                            st_ = {}

                            def scores():
                                stg = stpool.tile([128, 1024], f32, tag="st",
                                                  name="stg")
                                st_["stg"] = stg
                                for hh in range(2):
                                    nc.tensor.matmul(
                                        stg[:, hh * 512:hh * 512 + w],
                                        kt[j][kc][hh * 64:(hh + 1) * 64,
                                                  ko:ko + 128],
                                        qt[j][c][hh * 64:(hh + 1) * 64,
                                                 qoff:qoff + w],
                                        start=True, stop=True)

                            def pass1():
                                y16 = ypool.tile([128, 1024], f16, tag="y16",
                                                 name="y16")
                                st_["y16"] = y16
                                ydst = y16[:].rearrange(
                                    "p (hh w) -> p hh w", hh=2)[:, :, 0:w]
                                src = st_["stg"][:].rearrange(
                                    "p (hh w) -> p hh w", hh=2)[:, :, 0:w]
                                eng = ew_pick(2 * w * 0.833, 2 * w * 1.04)
                                if eng == "act":
                                    nc.scalar.activation(ydst, src, COPY,
                                                         bias=C1, scale=K1)
                                else:
                                    nc.vector.tensor_scalar(ydst, src, K1, C1,
                                                            MULT, ADD)

                            def pass2():
                                et = epool.tile([128, 1024], i16, tag="e",
                                                name=f"e{c}_{j}_{t}")
                                st_["e"] = et
                                edst = et[:].rearrange(
                                    "p (hh w) -> p hh w", hh=2)[:, :, 0:w]
                                ysrc = st_["y16"][:].bitcast(i16).rearrange(
                                    "p (hh w) -> p hh w", hh=2)[:, :, 0:w]
                                if r < 0:
                                    eng = ew_pick(2 * w * 0.833, 2 * w * 0.52)
                                    if eng == "act":
                                        nc.scalar.activation(
                                            edst, ysrc, COPY,
                                            bias=-25600.0 * 32.0, scale=32.0)
                                    else:
                                        nc.vector.tensor_scalar(
                                            edst, ysrc, 25600, M32, SUB, MULT)
                                else:
                                    mv = msk[:].rearrange(
                                        "p (hh w) -> p hh w", hh=2)[:, :, 0:w]
                                    nc.vector.scalar_tensor_tensor(
                                        edst, ysrc, 25600, mv, SUB, MULT)
                                    ew_book("dve", 2 * w * 0.52 + 180)

                            def pv():
                                ev = st_["e"][:].bitcast(f16).rearrange(
                                    "p (hh w) -> p hh w", hh=2)
                                for hh in range(2):
                                    h = 2 * j + hh
                                    nc.tensor.matmul(
                                        pz[hh][:, qoff:qoff + w],
                                        vb[:, h, t, 0:65], ev[:, hh, 0:w],
                                        start=(t == 0), stop=(t == nt - 1))

                            return {"scores": scores, "pass1": pass1,
                                    "pass2": pass2, "pv": pv}

                        units.append(mk())

                    # after the last PV of this j: the normalization chain
                    last_u = len(units) - 1

                    def norm_a(c=c, j=j, pz=pz, jstate=jstate):
                        rrA = rrpool.tile([1, 512], f32, tag="rr", name="rrA")
                        rrB = rrpool.tile([1, 512], f32, tag="rr", name="rrB")
                        nc.vector.reciprocal(rrA[:], pz[0][64:65, :])
                        nc.vector.reciprocal(rrB[:], pz[1][64:65, :])
                        ew_book("dve", 2 * 512 * 1.04 + 360)
                        bcs = bcpool.tile([128, 512], f32, tag="bcs",
                                          name="bcs")
                        jstate["bcs"] = bcs
                        # partition-broadcast by doubling DMAs (rows 0-63
                        # <- rrA, rows 64-127 <- rrB)
                        nc.gpsimd.dma_start(bcs[0:1, :], rrA[:])
                        nc.gpsimd.dma_start(bcs[64:65, :], rrB[:])
                        p = 1
                        while p < 64:
                            nc.gpsimd.dma_start(bcs[p:2 * p, :], bcs[0:p, :])
                            nc.gpsimd.dma_start(bcs[64 + p:64 + 2 * p, :],
                                                bcs[64:64 + p, :])
                            p *= 2

                    def norm_b(c=c, j=j, pz=pz, jstate=jstate):
                        bcs = jstate["bcs"]
                        zv = zh_cs[c][:].rearrange("p (q j m) -> p q j m",
                                                   q=4, j=NP)
                        pzv0 = pz[0][0:64, :].rearrange("p (q m) -> p q m", q=4)
                        pzv1 = pz[1][0:64, :].rearrange("p (q m) -> p q m", q=4)
                        bv0 = bcs[0:64, :].rearrange("p (q m) -> p q m", q=4)
                        bv1 = bcs[64:128, :].rearrange("p (q m) -> p q m", q=4)
                        nc.vector.tensor_mul(zv[0:64, :, j, :], pzv0, bv0)
                        nc.vector.tensor_mul(zv[64:128, :, j, :], pzv1, bv1)
                        ew_book("dve", 1024 * 1.04 + 360)

                    post.setdefault(last_u, []).append(norm_a)
                    post.setdefault(last_u + 2, []).append(norm_b)

                # chunk tail: 8 W_O units, deferred into the next chunk
                def wo_unit(c, qs, half):
                    def go():
                        zv = zh_cs[c][:].rearrange("p (q j m) -> p q j m",
                                                   q=4, j=NP)
                        po = mspool.tile([128, 512], f32, tag="ms", name="po")
                        for j in range(NP):
                            nc.tensor.matmul(
                                po[:, 0:384], zv[:, qs, j, :],
                                wo_sb[:, j * 768 + half * 384:
                                      j * 768 + (half + 1) * 384],
                                start=(j == 0), stop=(j == NP - 1))
                        osb = opool.tile([128, 384], f32, tag="osb", name="osb")
                        eng = ew_pick(384 * 0.833, 384 * 1.04)
                        if eng == "act":
                            nc.scalar.copy(osb[:], po[:, 0:384])
                        else:
                            nc.vector.tensor_copy(osb[:], po[:, 0:384])
                        row = c * 512 + qs * 128
                        hsl = slice(half * 384, (half + 1) * 384)
                        q_e = nc.sync if (qs + half) % 2 == 0 else nc.gpsimd
                        q_e.dma_start(out[row:row + 128, hsl], osb[:])
                    return go

                base = len(units) - 1
                for k8 in range(8):
                    post.setdefault(base + 3 + k8, []).append(
                        wo_unit(c, k8 // 2, k8 % 2))

            # filler: projections for chunks 1..3 (chunk 0 done in prologue)
            def filler_units():
                for c in range(1, QC):
                    yield from proj_units(c, mspool)

            filler = iter(filler_units())
            # schedule of per-(c) unit counts to pace projections: proj for
            # chunk c+1 must complete during chunk c's units.
            NU = len(units)
            pending_post = []
            fill_budget = {}
            # chunk unit ranges
            ranges = []
            u0 = 0
            for c in range(QC):
                n = (4 * c + 4) * NP
                ranges.append((u0, u0 + n))
                u0 += n

            fired = [0]

            def fire_slot(u):
                todo = post.pop(u, None)
                if todo:
                    pending_post.extend(todo)
                if pending_post:
                    pending_post.pop(0)()
                    return
                f = next(filler, None)
                if f is not None:
                    f()

            PIPE1, PIPE2, PIPE3 = 1, 2, 3
            for u in range(NU + PIPE3):
                if u < NU:
                    units[u]["scores"]()
                if 0 <= u - PIPE1 < NU:
                    units[u - PIPE1]["pass1"]()
                if 0 <= u - PIPE2 < NU:
                    units[u - PIPE2]["pass2"]()
                if 0 <= u - PIPE3 < NU:
                    units[u - PIPE3]["pv"]()
                fire_slot(u - PIPE3)
            # drain remaining slots
            for u in range(NU, NU + 32):
                todo = post.pop(u - PIPE3, None)
                if todo:
                    pending_post.extend(todo)
            while pending_post:
                pending_post.pop(0)()
            for f in filler:
                f()
            assert not post, f"unfired post slots: {list(post)}"

    _split_drain_waits(nc, mybir)
    return nc


_nc_cache = None


def _prep_core_inputs(x, W_Q, W_K, W_V, W_O, b_Q, b_K, core):
    import ml_dtypes
    bf16 = ml_dtypes.bfloat16
    b, g = core // G, core % G
    hs = slice(g * HPG, (g + 1) * HPG)

    xb = np.asarray(x[b], np.float32)               # [2048, 768]
    xT = np.ascontiguousarray(xb.T).astype(bf16)

    def wqk(W):
        # [p, a, (j hh d)] <- W[2j+hh, a*128+p, d]
        w = np.asarray(W, np.float32)[hs]           # [6, 768, 64]
        return np.ascontiguousarray(
            w.reshape(3, 2, 6, 128, 64).transpose(3, 2, 0, 1, 4)
            .reshape(128, -1)).astype(bf16)

    def wvp(W):
        # [p, a, (h d)] <- W[h, a*128+p, d]
        w = np.asarray(W, np.float32)[hs]
        return np.ascontiguousarray(
            w.reshape(6, 6, 128, 64).transpose(2, 1, 0, 3)
            .reshape(128, -1)).astype(bf16)

    # wo: [p=(hh*64+d), (j dcol)] <- W_O[2j+hh, d, dcol]
    wo = np.asarray(W_O, np.float32)[hs]            # [6, 64, 768]
    wod = np.ascontiguousarray(
        wo.reshape(3, 2, 64, 768).transpose(1, 2, 0, 3)
        .reshape(128, -1)).astype(bf16)

    def bp(bias):
        bb = np.asarray(bias, np.float32)[hs]       # [6, 64]
        return np.ascontiguousarray(
            bb.reshape(3, 2, 64).transpose(1, 2, 0).reshape(128, 3))

    p = np.arange(128)[:, None]
    q = np.arange(512)[None, :]
    m1 = np.where((q < 128) & (q < p), 0, M32).astype(np.int16)
    mA = np.concatenate([m1, m1], 1)

    return {
        "xT": xT, "wq": wqk(W_Q), "wk": wqk(W_K), "wv": wvp(W_V),
        "wo": wod, "bqd": bp(b_Q), "bkd": bp(b_K), "mskd": mA,
        "ones2_d": np.ones((128, 128), np.float32),
    }


def kernel(normalized_resid_pre, W_Q, W_K, W_V, W_O, b_Q, b_K, b_V, b_O):
    from concourse.bass_utils import run_bass_kernel_spmd

    global _nc_cache
    if _nc_cache is None:
        _nc_cache = build_program()
    nc = _nc_cache

    x = np.asarray(normalized_resid_pre, np.float32)
    in_maps = [_prep_core_inputs(x, W_Q, W_K, W_V, W_O, b_Q, b_K, c)
               for c in range(N_CORES)]

    res = run_bass_kernel_spmd(nc, in_maps, core_ids=list(range(N_CORES)))
    out = np.zeros((B, S, D), np.float32)
    for c in range(N_CORES):
        out[c // G] += np.asarray(res.results[c]["out"], np.float32)
    # bias folds: b_V rides through softmax rows (sum to 1) into W_O
    out += np.asarray(b_O, np.float32)
    out += np.einsum("nh,nhd->d", np.asarray(b_V, np.float32),
                     np.asarray(W_O, np.float32))
    return out


# revision 10
# speedup vs baseline: 1.2810x; 1.2810x over previous
"""Causal multi-head attention on 8 Trainium2 NeuronCores (bf16/fp16).

Sharding: core c -> (batch b = c//2, head-group g = c%2 of 6 heads).
Host sums the two half-head partial outputs per batch.

v5 design (all-16-bit matmuls; exp as a 2-op fp16 bit trick; deep
software pipeline so the PE never waits on the exp chain):
  - projections: plain bf16 matmuls (K=128 x 6 k-tiles)
  - scores: bf16 K=64 matmuls; the two heads of a j-pair run in
    concurrent PE row groups (0,0)/(64,0) -> ~2x, writing one
    [128, 1024] staging tile ([head A | head B]); causal trim per tile
  - exp: 2-op Schraudolph on fp16 bits:
      pass1: y = fp16(S*k1 + c1)  (k1=32*log2e/8, c1=15*32+1024) so
             round-to-int is free in fp16's [1024,2048) window
      pass2: int16 (bits(y) - 25600) * 32 = fp16 bits of 2^t with a
             linear 5-bit mantissa; boundary tiles multiply by a
             {32|0} triangle mask instead (exact zeros)
    split between ACT (float path) and DVE (int16 2x) by booked cost
  - pipeline ring: scores(t) | pass1(t-1) | pass2(t-2) | PV(t-3), so PV
    never stalls the in-order PE queue behind the exp chain
  - PV: fp16 e x fp16 V_aug [128, 65] (ones col -> denominator row 64)
  - normalize: DVE reciprocal straight off the PSUM denom rows; the
    recip rows are partition-broadcast by doubling SBUF DMAs (no PE
    matmul on the recip path); TT mul -> zh bf16
  - W_O: plain bf16, f32 out via ACT/DVE copy + DMA
"""

import numpy as np

B = 4
S = 2048
D = 768
NH = 12
DH = 64
G = 2            # head groups (tensor parallel)
HPG = NH // G    # heads per group = 6
NP = HPG // 2    # head pairs per group = 3
ST = S // 128    # 16 s-tiles
QC = S // 512    # 4 q-chunks
N_CORES = 8
VS = 66          # per-(head,tile) stride in v_big (64 V + 1 ones + 1 pad)

FS = 32.0                                  # exponent fraction scale
K1 = FS * 1.4426950408889634 / 8.0         # pass1 scale
C1 = 15.0 * FS + 1024.0                    # pass1 bias
M32 = 32                                   # pass2 multiplier


def _split_drain_waits(nc, mybir, max_waits=1):
    """Walrus accepts one sync wait per instruction; hoist extras onto
    NoOps on the same engine (program order keeps the gating)."""
    for f in nc.m.functions:
        for bb in f.blocks:
            newlist = []
            for ins in bb.instructions:
                si = ins.sync_info
                if si is not None and si.on_wait and len(si.on_wait) > max_waits:
                    waits = list(si.on_wait)
                    for i, w in enumerate(waits[:-max_waits]):
                        d = mybir.InstNoOp(name=f"{ins.name}-sw{i}", ins=[], outs=[])
                        d.engine = ins.engine
                        d.sync_info = mybir.SyncInfo(on_wait=[w], on_update=[])
                        newlist.append(d)
                    ins.sync_info = mybir.SyncInfo(
                        on_wait=list(waits[-max_waits:]), on_update=list(si.on_update)
                    )
                newlist.append(ins)
            try:
                bb.instructions = newlist
            except Exception:
                bb.instructions.clear()
                bb.instructions.extend(newlist)


def build_program():
    import concourse.bass as bass
    import concourse.mybir as mybir
    import concourse.tile as tile
    from contextlib import ExitStack

    f32 = mybir.dt.float32
    bf16 = mybir.dt.bfloat16
    f16 = mybir.dt.float16
    i16 = mybir.dt.int16
    MULT = mybir.AluOpType.mult
    SUB = mybir.AluOpType.subtract
    ADD = mybir.AluOpType.add
    COPY = mybir.ActivationFunctionType.Copy
    IDENT = mybir.ActivationFunctionType.Identity

    nc = bass.Bass("TRN2")
    xT = nc.dram_tensor("xT", [D, S], bf16, kind="ExternalInput")
    wq = nc.dram_tensor("wq", [128, 6 * 384], bf16, kind="ExternalInput")
    wk = nc.dram_tensor("wk", [128, 6 * 384], bf16, kind="ExternalInput")
    wv = nc.dram_tensor("wv", [128, 6 * 384], bf16, kind="ExternalInput")
    wo = nc.dram_tensor("wo", [128, 3 * 768], bf16, kind="ExternalInput")
    bqd = nc.dram_tensor("bqd", [128, 3], f32, kind="ExternalInput")
    bkd = nc.dram_tensor("bkd", [128, 3], f32, kind="ExternalInput")
    mskd = nc.dram_tensor("mskd", [128, 1024], i16, kind="ExternalInput")
    ones2_d = nc.dram_tensor("ones2_d", [128, 128], f32, kind="ExternalInput")
    ones1_d = nc.dram_tensor("ones1_d", [1, 128], bf16, kind="ExternalInput")
    out = nc.dram_tensor("out", [S, D], f32, kind="ExternalOutput")

    ew_load = {"act": 0.0, "dve": 0.0}

    def ew_pick(act_ns, dve_ns):
        ca = ew_load["act"] + act_ns + 180
        cd = ew_load["dve"] + dve_ns + 180
        if ca <= cd:
            ew_load["act"] = ca
            return "act"
        ew_load["dve"] = cd
        return "dve"

    def ew_book(which, ns):
        ew_load[which] += ns

    with tile.TileContext(nc) as tc:
        with ExitStack() as _ctx:
            _e = _ctx.enter_context
            _e(nc.allow_low_precision(reason="bf16/fp16 attention pipeline"))
            wpool = _e(tc.tile_pool(name="weights", bufs=1))
            xpool = _e(tc.tile_pool(name="xt", bufs=6))
            qkpool = _e(tc.tile_pool(name="qk", bufs=2 * NP * QC))
            vpool = _e(tc.tile_pool(name="v", bufs=1))
            ypool = _e(tc.tile_pool(name="y16", bufs=2))
            epool = _e(tc.tile_pool(name="e", bufs=3))
            zhpool = _e(tc.tile_pool(name="zh", bufs=2))
            rrpool = _e(tc.tile_pool(name="rr", bufs=2))
            bcpool = _e(tc.tile_pool(name="bcs", bufs=2))
            opool = _e(tc.tile_pool(name="osb", bufs=3))
            stpool = _e(tc.tile_pool(name="st", bufs=2, space="PSUM"))
            pzpool = _e(tc.tile_pool(name="pz", bufs=3, space="PSUM"))
            mspool = _e(tc.tile_pool(name="ms", bufs=1, space="PSUM"))

            # ---- small constants ----
            bq_sb = wpool.tile([128, NP], f32, tag="bq")
            nc.sync.dma_start(bq_sb[:], bqd[:])
            bk_sb = wpool.tile([128, NP], f32, tag="bk")
            nc.sync.dma_start(bk_sb[:], bkd[:])
            msk = wpool.tile([128, 1024], i16, tag="msk")
            nc.gpsimd.dma_start(msk[:], mskd[:])
            ones2 = wpool.tile([128, 128], f32, tag="ones2")
            nc.gpsimd.dma_start(ones2[:], ones2_d[:])
            ones_bf = wpool.tile([1, 128], bf16, tag="ones1")
            nc.sync.dma_start(ones_bf[:], ones1_d[:])

            # ---- PE warmup (p-state) while input DMAs land ----
            wu = mspool.tile([128, 512], f32, tag="ms", name="wu")
            for _ in range(20):
                nc.tensor.matmul(wu[:, 0:128], ones2[:], ones2[:],
                                 start=True, stop=True)

            # ---- bulk inputs over two DMA queues ----
            xt = [xpool.tile([128, S], bf16, tag="xt", name=f"xt{a}")
                  for a in range(6)]
            for a in range(6):
                half = S // 2
                eng = nc.sync if a % 2 == 0 else nc.gpsimd
                eng.dma_start(xt[a][:, 0:half], xT[a * 128:(a + 1) * 128, 0:half])
                eng2 = nc.gpsimd if a % 2 == 0 else nc.sync
                eng2.dma_start(xt[a][:, half:S], xT[a * 128:(a + 1) * 128, half:S])
            wq_sb = wpool.tile([128, 6 * 384], bf16, tag="wq")
            nc.sync.dma_start(wq_sb[:], wq[:])
            wk_sb = wpool.tile([128, 6 * 384], bf16, tag="wk")
            nc.gpsimd.dma_start(wk_sb[:], wk[:])
            wv_sb = wpool.tile([128, 6 * 384], bf16, tag="wv")
            nc.sync.dma_start(wv_sb[:], wv[:])
            wo_sb = wpool.tile([128, 3 * 768], bf16, tag="wo")
            nc.gpsimd.dma_start(wo_sb[:], wo[:])

            # ---- persistent activation tiles ----
            qt = [[qkpool.tile([128, 512], bf16, tag="qk", name=f"qt{j}_{c}")
                   for c in range(QC)] for j in range(NP)]
            kt = [[qkpool.tile([128, 512], bf16, tag="qk", name=f"kt{j}_{c}")
                   for c in range(QC)] for j in range(NP)]
            v_big = vpool.tile([128, HPG * 16 * VS], f16, tag="v")
            vb = v_big[:].rearrange("p (h t m) -> p h t m", h=HPG, t=16)
            nc.gpsimd.memset(vb[:, :, :, 64:65], 1.0)

            zh_cs = [zhpool.tile([128, 4 * NP * 128], bf16, tag="zh",
                                 name=f"zh{c}") for c in range(QC)]

            def emit_proj_qk(c, j, which, pool):
                w_sb, b_sb, dst = ((wq_sb, bq_sb, qt[j][c]) if which == 0
                                   else (wk_sb, bk_sb, kt[j][c]))
                ps = pool.tile([128, 512], f32,
                               tag="st" if pool is stpool else "ms",
                               name="psqk")
                for a in range(6):
                    nc.tensor.matmul(
                        ps[:], w_sb[:, a * 384 + j * 128:a * 384 + (j + 1) * 128],
                        xt[a][:, c * 512:(c + 1) * 512],
                        start=(a == 0), stop=(a == 5))
                eng = ew_pick(512 * 0.833, 512 * 1.04)
                if eng == "act":
                    nc.scalar.activation(dst[:], ps[:], IDENT,
                                         bias=b_sb[:, j:j + 1], scale=1.0)
                else:
                    nc.vector.tensor_scalar_add(dst[:], ps[:], b_sb[:, j:j + 1])

            def emit_proj_v(st, pool):
                ps = pool.tile([128, 512], f32,
                               tag="st" if pool is stpool else "ms",
                               name="psv")
                for a in range(6):
                    nc.tensor.matmul(
                        ps[:, 0:384], xt[a][:, st * 128:(st + 1) * 128],
                        wv_sb[:, a * 384:(a + 1) * 384],
                        start=(a == 0), stop=(a == 5))
                dst = vb[:, :, st, 0:64]
                src = ps[:, 0:384].rearrange("p (h d) -> p h d", h=HPG)
                eng = ew_pick(384 * 0.833, 384 * 1.04)
                if eng == "act":
                    nc.scalar.copy(dst, src)
                else:
                    nc.vector.tensor_copy(dst, src)

            def proj_units(c, pool):
                if c >= QC:
                    return
                for j in range(NP):
                    yield lambda j=j: emit_proj_qk(c, j, 0, pool)
                    yield lambda j=j: emit_proj_qk(c, j, 1, pool)
                for st in range(4 * c, 4 * c + 4):
                    yield lambda st=st: emit_proj_v(st, pool)

            # prologue: chunk-0 projections through the (idle) stg pool
            for u in proj_units(0, stpool):
                u()

            # ---------- global software-pipelined attention ----------
            # unit = one k-tile of one (c, j). Stages within a unit step:
            # scores(u) | pass1(u-1) | pass2(u-2) | PV(u-3) | slot
            units = []      # list of dicts with stage closures
            post = {}       # unit index -> list of thunks (after PV stage)

            for c in range(QC):
                nt = 4 * c + 4
                for j in range(NP):
                    pz = [pzpool.tile([65, 512], f32, tag="pz",
                                      name=f"pz{c}_{j}_{hh}") for hh in range(2)]
                    jstate = {}

                    for t in range(nt):
                        r = t - 4 * c
                        qoff = max(r, 0) * 128
                        w = 512 - qoff
                        kc, ko = t // 4, (t % 4) * 128

                        def mk(c=c, j=j, t=t, r=r, qoff=qoff, w=w, kc=kc,
                               ko=ko, pz=pz, jstate=jstate, nt=nt):
                            st_ = {}

                            def scores():
                                stg = stpool.tile([128, 1024], f32, tag="st",
                                                  name="stg")
                                st_["stg"] = stg
                                for hh in range(2):
                                    nc.tensor.matmul(
                                        stg[:, hh * 512:hh * 512 + w],
                                        kt[j][kc][hh * 64:(hh + 1) * 64,
                                                  ko:ko + 128],
                                        qt[j][c][hh * 64:(hh + 1) * 64,
                                                 qoff:qoff + w],
                                        start=True, stop=True)

                            def pass1():
                                y16 = ypool.tile([128, 1024], f16, tag="y16",
                                                 name="y16")
                                st_["y16"] = y16
                                ydst = y16[:].rearrange(
                                    "p (hh w) -> p hh w", hh=2)[:, :, 0:w]
                                src = st_["stg"][:].rearrange(
                                    "p (hh w) -> p hh w", hh=2)[:, :, 0:w]
                                eng = ew_pick(2 * w * 0.833, 2 * w * 1.04)
                                if eng == "act":
                                    nc.scalar.activation(ydst, src, COPY,
                                                         bias=C1, scale=K1)
                                else:
                                    nc.vector.tensor_scalar(ydst, src, K1, C1,
                                                            MULT, ADD)

                            def pass2():
                                et = epool.tile([128, 1024], i16, tag="e",
                                                name=f"e{c}_{j}_{t}")
                                st_["e"] = et
                                edst = et[:].rearrange(
                                    "p (hh w) -> p hh w", hh=2)[:, :, 0:w]
                                ysrc = st_["y16"][:].bitcast(i16).rearrange(
                                    "p (hh w) -> p hh w", hh=2)[:, :, 0:w]
                                if r < 0:
                                    eng = ew_pick(2 * w * 0.833, 2 * w * 0.52)
                                    if eng == "act":
                                        nc.scalar.activation(
                                            edst, ysrc, COPY,
                                            bias=-25600.0 * 32.0, scale=32.0)
                                    else:
                                        nc.vector.tensor_scalar(
                                            edst, ysrc, 25600, M32, SUB, MULT)
                                else:
                                    mv = msk[:].rearrange(
                                        "p (hh w) -> p hh w", hh=2)[:, :, 0:w]
                                    nc.vector.scalar_tensor_tensor(
                                        edst, ysrc, 25600, mv, SUB, MULT)
                                    ew_book("dve", 2 * w * 0.52 + 180)

                            def pv():
                                ev = st_["e"][:].bitcast(f16).rearrange(
                                    "p (hh w) -> p hh w", hh=2)
                                for hh in range(2):
                                    h = 2 * j + hh
                                    nc.tensor.matmul(
                                        pz[hh][:, qoff:qoff + w],
                                        vb[:, h, t, 0:65], ev[:, hh, 0:w],
                                        start=(t == 0), stop=(t == nt - 1))

                            return {"scores": scores, "pass1": pass1,
                                    "pass2": pass2, "pv": pv}

                        units.append(mk())

                    # after the last PV of this j: the normalization chain
                    last_u = len(units) - 1

                    def norm_a(c=c, j=j, pz=pz, jstate=jstate):
                        # denom rows -> SBUF, spread over partitions, small
                        # reciprocal, bf16, back to two rows
                        dn = rrpool.tile([1, 1024], f32, tag="dn", name="dn")
                        for hh in range(2):
                            eng = ew_pick(512 * 0.833, 512 * 1.04)
                            if eng == "act":
                                nc.scalar.copy(dn[:, hh * 512:(hh + 1) * 512],
                                               pz[hh][64:65, :])
                            else:
                                nc.vector.tensor_copy(
                                    dn[:, hh * 512:(hh + 1) * 512],
                                    pz[hh][64:65, :])
                        dnp = rrpool.tile([128, 8], f32, tag="dnp", name="dnp")
                        nc.gpsimd.dma_start(dnp[:], dn[:])
                        rp = rrpool.tile([128, 8], bf16, tag="rp", name="rp")
                        rpf = rrpool.tile([128, 8], f32, tag="rpf", name="rpf")
                        nc.vector.reciprocal(rpf[:], dnp[:])
                        nc.vector.tensor_copy(rp[:], rpf[:])
                        ew_book("dve", 400)
                        rrA = rrpool.tile([1, 512], bf16, tag="rr", name="rrA")
                        rrB = rrpool.tile([1, 512], bf16, tag="rr", name="rrB")
                        nc.gpsimd.dma_start(rrA[:], rp[0:64, :])
                        nc.gpsimd.dma_start(rrB[:], rp[64:128, :])
                        jstate["rr"] = (rrA, rrB)

                    def norm_b(c=c, j=j, pz=pz, jstate=jstate):
                        rrA, rrB = jstate["rr"]
                        bcp = mspool.tile([128, 512], f32, tag="ms",
                                          name="bcp")
                        nc.tensor.matmul(bcp[0:64, :], ones_bf[:, 0:64],
                                         rrA[:], start=True, stop=True)
                        nc.tensor.matmul(bcp[64:128, :], ones_bf[:, 0:64],
                                         rrB[:], start=True, stop=True)
                        bcs = bcpool.tile([128, 512], bf16, tag="bcs",
                                          name="bcs")
                        eng = ew_pick(512 * 0.833, 512 * 1.04)
                        if eng == "act":
                            nc.scalar.copy(bcs[:], bcp[:])
                        else:
                            nc.vector.tensor_copy(bcs[:], bcp[:])
                        zv = zh_cs[c][:].rearrange("p (q j m) -> p q j m",
                                                   q=4, j=NP)
                        pzv0 = pz[0][0:64, :].rearrange("p (q m) -> p q m", q=4)
                        pzv1 = pz[1][0:64, :].rearrange("p (q m) -> p q m", q=4)
                        bv0 = bcs[0:64, :].rearrange("p (q m) -> p q m", q=4)
                        bv1 = bcs[64:128, :].rearrange("p (q m) -> p q m", q=4)
                        nc.vector.tensor_mul(zv[0:64, :, j, :], pzv0, bv0)
                        nc.vector.tensor_mul(zv[64:128, :, j, :], pzv1, bv1)
                        ew_book("dve", 1024 * 1.04 + 360)

                    post.setdefault(last_u, []).append(norm_a)
                    post.setdefault(last_u + 2, []).append(norm_b)

                # chunk tail: 8 W_O units, deferred into the next chunk
                def wo_unit(c, qs, half):
                    def go():
                        zv = zh_cs[c][:].rearrange("p (q j m) -> p q j m",
                                                   q=4, j=NP)
                        po = mspool.tile([128, 512], f32, tag="ms", name="po")
                        for j in range(NP):
                            nc.tensor.matmul(
                                po[:, 0:384], zv[:, qs, j, :],
                                wo_sb[:, j * 768 + half * 384:
                                      j * 768 + (half + 1) * 384],
                                start=(j == 0), stop=(j == NP - 1))
                        osb = opool.tile([128, 384], f32, tag="osb", name="osb")
                        eng = ew_pick(384 * 0.833, 384 * 1.04)
                        if eng == "act":
                            nc.scalar.copy(osb[:], po[:, 0:384])
                        else:
                            nc.vector.tensor_copy(osb[:], po[:, 0:384])
                        row = c * 512 + qs * 128
                        hsl = slice(half * 384, (half + 1) * 384)
                        q_e = nc.sync if (qs + half) % 2 == 0 else nc.gpsimd
                        q_e.dma_start(out[row:row + 128, hsl], osb[:])
                    return go

                base = len(units) - 1
                for k8 in range(8):
                    post.setdefault(base + 3 + k8, []).append(
                        wo_unit(c, k8 // 2, k8 % 2))

            # filler: projections for chunks 1..3 (chunk 0 done in prologue)
            def filler_units():
                for c in range(1, QC):
                    yield from proj_units(c, mspool)

            filler = iter(filler_units())
            # schedule of per-(c) unit counts to pace projections: proj for
            # chunk c+1 must complete during chunk c's units.
            NU = len(units)
            pending_post = []
            fill_budget = {}
            # chunk unit ranges
            ranges = []
            u0 = 0
            for c in range(QC):
                n = (4 * c + 4) * NP
                ranges.append((u0, u0 + n))
                u0 += n

            fired = [0]

            def fire_slot(u):
                todo = post.pop(u, None)
                if todo:
                    pending_post.extend(todo)
                if pending_post:
                    pending_post.pop(0)()
                    return
                f = next(filler, None)
                if f is not None:
                    f()

            PIPE1, PIPE2, PIPE3 = 1, 2, 3
            for u in range(NU + PIPE3):
                if u < NU:
                    units[u]["scores"]()
                if 0 <= u - PIPE1 < NU:
                    units[u - PIPE1]["pass1"]()
                if 0 <= u - PIPE2 < NU:
                    units[u - PIPE2]["pass2"]()
                if 0 <= u - PIPE3 < NU:
                    units[u - PIPE3]["pv"]()
                fire_slot(u - PIPE3)
            # drain remaining slots
            for u in range(NU, NU + 32):
                todo = post.pop(u - PIPE3, None)
                if todo:
                    pending_post.extend(todo)
            while pending_post:
                pending_post.pop(0)()
            for f in filler:
                f()
            assert not post, f"unfired post slots: {list(post)}"

    _split_drain_waits(nc, mybir)
    return nc


_nc_cache = None


def _prep_core_inputs(x, W_Q, W_K, W_V, W_O, b_Q, b_K, core):
    import ml_dtypes
    bf16 = ml_dtypes.bfloat16
    b, g = core // G, core % G
    hs = slice(g * HPG, (g + 1) * HPG)

    xb = np.asarray(x[b], np.float32)               # [2048, 768]
    xT = np.ascontiguousarray(xb.T).astype(bf16)

    def wqk(W):
        # [p, a, (j hh d)] <- W[2j+hh, a*128+p, d]
        w = np.asarray(W, np.float32)[hs]           # [6, 768, 64]
        return np.ascontiguousarray(
            w.reshape(3, 2, 6, 128, 64).transpose(3, 2, 0, 1, 4)
            .reshape(128, -1)).astype(bf16)

    def wvp(W):
        # [p, a, (h d)] <- W[h, a*128+p, d]
        w = np.asarray(W, np.float32)[hs]
        return np.ascontiguousarray(
            w.reshape(6, 6, 128, 64).transpose(2, 1, 0, 3)
            .reshape(128, -1)).astype(bf16)

    # wo: [p=(hh*64+d), (j dcol)] <- W_O[2j+hh, d, dcol]
    wo = np.asarray(W_O, np.float32)[hs]            # [6, 64, 768]
    wod = np.ascontiguousarray(
        wo.reshape(3, 2, 64, 768).transpose(1, 2, 0, 3)
        .reshape(128, -1)).astype(bf16)

    def bp(bias):
        bb = np.asarray(bias, np.float32)[hs]       # [6, 64]
        return np.ascontiguousarray(
            bb.reshape(3, 2, 64).transpose(1, 2, 0).reshape(128, 3))

    p = np.arange(128)[:, None]
    q = np.arange(512)[None, :]
    m1 = np.where((q < 128) & (q < p), 0, M32).astype(np.int16)
    mA = np.concatenate([m1, m1], 1)

    return {
        "xT": xT, "wq": wqk(W_Q), "wk": wqk(W_K), "wv": wvp(W_V),
        "wo": wod, "bqd": bp(b_Q), "bkd": bp(b_K), "mskd": mA,
        "ones2_d": np.ones((128, 128), np.float32),
        "ones1_d": np.ones((1, 128), bf16),
    }


def kernel(normalized_resid_pre, W_Q, W_K, W_V, W_O, b_Q, b_K, b_V, b_O):
    from concourse.bass_utils import run_bass_kernel_spmd

    global _nc_cache
    if _nc_cache is None:
        _nc_cache = build_program()
    nc = _nc_cache

    x = np.asarray(normalized_resid_pre, np.float32)
    in_maps = [_prep_core_inputs(x, W_Q, W_K, W_V, W_O, b_Q, b_K, c)
               for c in range(N_CORES)]

    res = run_bass_kernel_spmd(nc, in_maps, core_ids=list(range(N_CORES)))
    out = np.zeros((B, S, D), np.float32)
    for c in range(N_CORES):
        out[c // G] += np.asarray(res.results[c]["out"], np.float32)
    # bias folds: b_V rides through softmax rows (sum to 1) into W_O
    out += np.asarray(b_O, np.float32)
    out += np.einsum("nh,nhd->d", np.asarray(b_V, np.float32),
                     np.asarray(W_O, np.float32))
    return out


# revision 12
# speedup vs baseline: 1.5493x; 1.2094x over previous
"""Causal multi-head attention on 8 Trainium2 NeuronCores (bf16/fp16).

Sharding: core c -> (batch b = c//2, head-group g = c%2 of 6 heads).
Host sums the two half-head partial outputs per batch.

v5 design (all-16-bit matmuls; exp as a 2-op fp16 bit trick; deep
software pipeline so the PE never waits on the exp chain):
  - projections: plain bf16 matmuls (K=128 x 6 k-tiles)
  - scores: bf16 K=64 matmuls; the two heads of a j-pair run in
    concurrent PE row groups (0,0)/(64,0) -> ~2x, writing one
    [128, 1024] staging tile ([head A | head B]); causal trim per tile
  - exp: ONE op: int16(S*KE + 15360) is the fp16 BIT PATTERN of
    2^n*(1+f) ~ exp(s) (Schraudolph with a 10-bit linear fraction);
    boundary tiles use an additive {15360|-1e6} triangle mask whose
    int16 saturation gives 0x8000 = -0.0 = exact zero pattern.
    Ops are split between ACT and DVE by booked cost
  - pipeline ring: scores(t) | exp(t-1) | PV(t-3), so PV never stalls
    the in-order PE queue behind the exp chain
  - PV: fp16 e x fp16 V_aug [128, 65] (ones col -> denominator row 64)
  - normalize: DVE reciprocal straight off the PSUM denom rows; the
    recip rows are partition-broadcast by doubling SBUF DMAs (no PE
    matmul on the recip path); TT mul -> zh bf16
  - W_O: plain bf16, f32 out via ACT/DVE copy + DMA
"""

import numpy as np

B = 4
S = 2048
D = 768
NH = 12
DH = 64
G = 2            # head groups (tensor parallel)
HPG = NH // G    # heads per group = 6
NP = HPG // 2    # head pairs per group = 3
ST = S // 128    # 16 s-tiles
QC = S // 512    # 4 q-chunks
N_CORES = 8
VS = 66          # per-(head,tile) stride in v_big (64 V + 1 ones + 1 pad)

KE = 1024.0 * 1.4426950408889634 / 8.0     # exp scale (PSUM -> fp16 bits)
CE = 15.0 * 1024.0                         # exp bias (fp16 exponent 15)
MASKNEG = -1.0e6                           # additive mask -> int16 saturate


def _split_drain_waits(nc, mybir, max_waits=1):
    """Walrus accepts one sync wait per instruction; hoist extras onto
    NoOps on the same engine (program order keeps the gating)."""
    for f in nc.m.functions:
        for bb in f.blocks:
            newlist = []
            for ins in bb.instructions:
                si = ins.sync_info
                if si is not None and si.on_wait and len(si.on_wait) > max_waits:
                    waits = list(si.on_wait)
                    for i, w in enumerate(waits[:-max_waits]):
                        d = mybir.InstNoOp(name=f"{ins.name}-sw{i}", ins=[], outs=[])
                        d.engine = ins.engine
                        d.sync_info = mybir.SyncInfo(on_wait=[w], on_update=[])
                        newlist.append(d)
                    ins.sync_info = mybir.SyncInfo(
                        on_wait=list(waits[-max_waits:]), on_update=list(si.on_update)
                    )
                newlist.append(ins)
            try:
                bb.instructions = newlist
            except Exception:
                bb.instructions.clear()
                bb.instructions.extend(newlist)


def build_program():
    import concourse.bass as bass
    import concourse.mybir as mybir
    import concourse.tile as tile
    from contextlib import ExitStack

    f32 = mybir.dt.float32
    bf16 = mybir.dt.bfloat16
    f16 = mybir.dt.float16
    i16 = mybir.dt.int16
    MULT = mybir.AluOpType.mult
    SUB = mybir.AluOpType.subtract
    ADD = mybir.AluOpType.add
    COPY = mybir.ActivationFunctionType.Copy
    IDENT = mybir.ActivationFunctionType.Identity

    nc = bass.Bass("TRN2")
    xT = nc.dram_tensor("xT", [D, S], bf16, kind="ExternalInput")
    wq = nc.dram_tensor("wq", [128, 6 * 384], bf16, kind="ExternalInput")
    wk = nc.dram_tensor("wk", [128, 6 * 384], bf16, kind="ExternalInput")
    wv = nc.dram_tensor("wv", [128, 6 * 384], bf16, kind="ExternalInput")
    wo = nc.dram_tensor("wo", [128, 3 * 768], bf16, kind="ExternalInput")
    bqd = nc.dram_tensor("bqd", [128, 3], f32, kind="ExternalInput")
    bkd = nc.dram_tensor("bkd", [128, 3], f32, kind="ExternalInput")
    mskd = nc.dram_tensor("mskd", [128, 1024], f32, kind="ExternalInput")
    ones2_d = nc.dram_tensor("ones2_d", [128, 128], f32, kind="ExternalInput")
    ones1_d = nc.dram_tensor("ones1_d", [1, 128], bf16, kind="ExternalInput")
    out = nc.dram_tensor("out", [S, D], f32, kind="ExternalOutput")

    ew_load = {"act": 0.0, "dve": 0.0}

    def ew_pick(act_ns, dve_ns):
        ca = ew_load["act"] + act_ns + 180
        cd = ew_load["dve"] + dve_ns + 180
        if ca <= cd:
            ew_load["act"] = ca
            return "act"
        ew_load["dve"] = cd
        return "dve"

    def ew_book(which, ns):
        ew_load[which] += ns

    with tile.TileContext(nc) as tc:
        with ExitStack() as _ctx:
            _e = _ctx.enter_context
            _e(nc.allow_low_precision(reason="bf16/fp16 attention pipeline"))
            wpool = _e(tc.tile_pool(name="weights", bufs=1))
            xpool = _e(tc.tile_pool(name="xt", bufs=6))
            qkpool = _e(tc.tile_pool(name="qk", bufs=2 * NP * QC))
            vpool = _e(tc.tile_pool(name="v", bufs=1))
            epool = _e(tc.tile_pool(name="e", bufs=3))
            zhpool = _e(tc.tile_pool(name="zh", bufs=2))
            rrpool = _e(tc.tile_pool(name="rr", bufs=2))
            bcpool = _e(tc.tile_pool(name="bcs", bufs=2))
            opool = _e(tc.tile_pool(name="osb", bufs=3))
            stpool = _e(tc.tile_pool(name="st", bufs=2, space="PSUM"))
            pzpool = _e(tc.tile_pool(name="pz", bufs=3, space="PSUM"))
            mspool = _e(tc.tile_pool(name="ms", bufs=1, space="PSUM"))

            # ---- small constants ----
            bq_sb = wpool.tile([128, NP], f32, tag="bq")
            nc.sync.dma_start(bq_sb[:], bqd[:])
            bk_sb = wpool.tile([128, NP], f32, tag="bk")
            nc.sync.dma_start(bk_sb[:], bkd[:])
            msk = wpool.tile([128, 1024], f32, tag="msk")
            nc.gpsimd.dma_start(msk[:], mskd[:])
            ones2 = wpool.tile([128, 128], f32, tag="ones2")
            nc.gpsimd.dma_start(ones2[:], ones2_d[:])
            ones_bf = wpool.tile([1, 128], bf16, tag="ones1")
            nc.sync.dma_start(ones_bf[:], ones1_d[:])

            # ---- PE warmup (p-state) while input DMAs land ----
            wu = mspool.tile([128, 512], f32, tag="ms", name="wu")
            for _ in range(20):
                nc.tensor.matmul(wu[:, 0:128], ones2[:], ones2[:],
                                 start=True, stop=True)

            # ---- bulk inputs over two DMA queues ----
            xt = [xpool.tile([128, S], bf16, tag="xt", name=f"xt{a}")
                  for a in range(6)]
            wq_sb = wpool.tile([128, 6 * 384], bf16, tag="wq")
            nc.sync.dma_start(wq_sb[:], wq[:])
            wk_sb = wpool.tile([128, 6 * 384], bf16, tag="wk")
            nc.gpsimd.dma_start(wk_sb[:], wk[:])
            for a in range(6):
                half = S // 2
                eng = nc.sync if a % 2 == 0 else nc.gpsimd
                eng.dma_start(xt[a][:, 0:half], xT[a * 128:(a + 1) * 128, 0:half])
            wv_sb = wpool.tile([128, 6 * 384], bf16, tag="wv")
            nc.sync.dma_start(wv_sb[:], wv[:])
            for a in range(6):
                half = S // 2
                eng2 = nc.gpsimd if a % 2 == 0 else nc.sync
                eng2.dma_start(xt[a][:, half:S], xT[a * 128:(a + 1) * 128, half:S])
            wo_sb = wpool.tile([128, 3 * 768], bf16, tag="wo")
            nc.gpsimd.dma_start(wo_sb[:], wo[:])

            # ---- persistent activation tiles ----
            qt = [[qkpool.tile([128, 512], bf16, tag="qk", name=f"qt{j}_{c}")
                   for c in range(QC)] for j in range(NP)]
            kt = [[qkpool.tile([128, 512], bf16, tag="qk", name=f"kt{j}_{c}")
                   for c in range(QC)] for j in range(NP)]
            v_big = vpool.tile([128, HPG * 16 * VS], f16, tag="v")
            vb = v_big[:].rearrange("p (h t m) -> p h t m", h=HPG, t=16)
            nc.gpsimd.memset(vb[:, :, :, 64:65], 1.0)

            zh_cs = [zhpool.tile([128, 4 * NP * 128], bf16, tag="zh",
                                 name=f"zh{c}") for c in range(QC)]

            def emit_proj_qk(c, j, which, pool):
                w_sb, b_sb, dst = ((wq_sb, bq_sb, qt[j][c]) if which == 0
                                   else (wk_sb, bk_sb, kt[j][c]))
                ps = pool.tile([128, 512], f32,
                               tag="st" if pool is stpool else "ms",
                               name="psqk")
                for a in range(6):
                    nc.tensor.matmul(
                        ps[:], w_sb[:, a * 384 + j * 128:a * 384 + (j + 1) * 128],
                        xt[a][:, c * 512:(c + 1) * 512],
                        start=(a == 0), stop=(a == 5))
                eng = ew_pick(512 * 0.833, 512 * 1.04)
                if eng == "act":
                    nc.scalar.activation(dst[:], ps[:], IDENT,
                                         bias=b_sb[:, j:j + 1], scale=1.0)
                else:
                    nc.vector.tensor_scalar_add(dst[:], ps[:], b_sb[:, j:j + 1])

            def emit_proj_v(st, pool):
                ps = pool.tile([128, 512], f32,
                               tag="st" if pool is stpool else "ms",
                               name="psv")
                for a in range(6):
                    nc.tensor.matmul(
                        ps[:, 0:384], xt[a][:, st * 128:(st + 1) * 128],
                        wv_sb[:, a * 384:(a + 1) * 384],
                        start=(a == 0), stop=(a == 5))
                dst = vb[:, :, st, 0:64]
                src = ps[:, 0:384].rearrange("p (h d) -> p h d", h=HPG)
                eng = ew_pick(384 * 0.833, 384 * 1.04)
                if eng == "act":
                    nc.scalar.copy(dst, src)
                else:
                    nc.vector.tensor_copy(dst, src)

            def proj_units(c, pool):
                if c >= QC:
                    return
                for j in range(NP):
                    yield lambda j=j: emit_proj_qk(c, j, 0, pool)
                    yield lambda j=j: emit_proj_qk(c, j, 1, pool)
                for st in range(4 * c, 4 * c + 4):
                    yield lambda st=st: emit_proj_v(st, pool)

            # prologue: chunk-0 projections through the (idle) stg pool
            for u in proj_units(0, stpool):
                u()

            # ---------- global software-pipelined attention ----------
            # unit = one k-tile of one (c, j). Stages within a unit step:
            # scores(u) | pass1(u-1) | pass2(u-2) | PV(u-3) | slot
            units = []      # list of dicts with stage closures
            post = {}       # unit index -> list of thunks (after PV stage)

            for c in range(QC):
                nt = 4 * c + 4
                for j in range(NP):
                    pz = [pzpool.tile([65, 512], f32, tag="pz",
                                      name=f"pz{c}_{j}_{hh}") for hh in range(2)]
                    jstate = {}

                    for t in range(nt):
                        r = t - 4 * c
                        qoff = max(r, 0) * 128
                        w = 512 - qoff
                        kc, ko = t // 4, (t % 4) * 128

                        def mk(c=c, j=j, t=t, r=r, qoff=qoff, w=w, kc=kc,
                               ko=ko, pz=pz, jstate=jstate, nt=nt):
                            st_ = {}

                            def scores():
                                stg = stpool.tile([128, 1024], f32, tag="st",
                                                  name="stg")
                                st_["stg"] = stg
                                for hh in range(2):
                                    nc.tensor.matmul(
                                        stg[:, hh * 512:hh * 512 + w],
                                        kt[j][kc][hh * 64:(hh + 1) * 64,
                                                  ko:ko + 128],
                                        qt[j][c][hh * 64:(hh + 1) * 64,
                                                 qoff:qoff + w],
                                        start=True, stop=True)

                            def expop():
                                et = epool.tile([128, 1024], i16, tag="e",
                                                name=f"e{c}_{j}_{t}")
                                st_["e"] = et
                                edst = et[:].rearrange(
                                    "p (hh w) -> p hh w", hh=2)[:, :, 0:w]
                                src = st_["stg"][:].rearrange(
                                    "p (hh w) -> p hh w", hh=2)[:, :, 0:w]
                                if r < 0:
                                    eng = ew_pick(2 * w * 0.833, 2 * w * 1.04)
                                    if eng == "act":
                                        nc.scalar.activation(
                                            edst, src, COPY,
                                            bias=CE, scale=KE)
                                    else:
                                        nc.vector.tensor_scalar(
                                            edst, src, KE, CE, MULT, ADD)
                                else:
                                    mv = msk[:].rearrange(
                                        "p (hh w) -> p hh w", hh=2)[:, :, 0:w]
                                    nc.vector.scalar_tensor_tensor(
                                        edst, src, KE, mv, MULT, ADD)
                                    ew_book("dve", 2 * w * 1.04 + 180)

                            def pv():
                                ev = st_["e"][:].bitcast(f16).rearrange(
                                    "p (hh w) -> p hh w", hh=2)
                                for hh in range(2):
                                    h = 2 * j + hh
                                    nc.tensor.matmul(
                                        pz[hh][:, qoff:qoff + w],
                                        vb[:, h, t, 0:65], ev[:, hh, 0:w],
                                        start=(t == 0), stop=(t == nt - 1))

                            return {"scores": scores, "exp": expop,
                                    "pv": pv}

                        units.append(mk())

                    # after the last PV of this j: the normalization chain
                    last_u = len(units) - 1

                    def norm_a(c=c, j=j, pz=pz, jstate=jstate):
                        # denom rows -> SBUF, spread over partitions, small
                        # reciprocal, bf16, back to two rows
                        dn = rrpool.tile([1, 1024], f32, tag="dn", name="dn")
                        for hh in range(2):
                            eng = ew_pick(512 * 0.833, 512 * 1.04)
                            if eng == "act":
                                nc.scalar.copy(dn[:, hh * 512:(hh + 1) * 512],
                                               pz[hh][64:65, :])
                            else:
                                nc.vector.tensor_copy(
                                    dn[:, hh * 512:(hh + 1) * 512],
                                    pz[hh][64:65, :])
                        dnp = rrpool.tile([128, 8], f32, tag="dnp", name="dnp")
                        nc.gpsimd.dma_start(dnp[:], dn[:])
                        rp = rrpool.tile([128, 8], bf16, tag="rp", name="rp")
                        rpf = rrpool.tile([128, 8], f32, tag="rpf", name="rpf")
                        nc.vector.reciprocal(rpf[:], dnp[:])
                        nc.vector.tensor_copy(rp[:], rpf[:])
                        ew_book("dve", 400)
                        rrA = rrpool.tile([1, 512], bf16, tag="rr", name="rrA")
                        rrB = rrpool.tile([1, 512], bf16, tag="rr", name="rrB")
                        nc.gpsimd.dma_start(rrA[:], rp[0:64, :])
                        nc.gpsimd.dma_start(rrB[:], rp[64:128, :])
                        jstate["rr"] = (rrA, rrB)

                    def norm_b(c=c, j=j, pz=pz, jstate=jstate):
                        rrA, rrB = jstate["rr"]
                        bcp = mspool.tile([128, 512], f32, tag="ms",
                                          name="bcp")
                        nc.tensor.matmul(bcp[0:64, :], ones_bf[:, 0:64],
                                         rrA[:], start=True, stop=True)
                        nc.tensor.matmul(bcp[64:128, :], ones_bf[:, 0:64],
                                         rrB[:], start=True, stop=True)
                        bcs = bcpool.tile([128, 512], bf16, tag="bcs",
                                          name="bcs")
                        eng = ew_pick(512 * 0.833, 512 * 1.04)
                        if eng == "act":
                            nc.scalar.copy(bcs[:], bcp[:])
                        else:
                            nc.vector.tensor_copy(bcs[:], bcp[:])
                        zv = zh_cs[c][:].rearrange("p (q j m) -> p q j m",
                                                   q=4, j=NP)
                        pzv0 = pz[0][0:64, :].rearrange("p (q m) -> p q m", q=4)
                        pzv1 = pz[1][0:64, :].rearrange("p (q m) -> p q m", q=4)
                        bv0 = bcs[0:64, :].rearrange("p (q m) -> p q m", q=4)
                        bv1 = bcs[64:128, :].rearrange("p (q m) -> p q m", q=4)
                        nc.vector.tensor_mul(zv[0:64, :, j, :], pzv0, bv0)
                        nc.vector.tensor_mul(zv[64:128, :, j, :], pzv1, bv1)
                        ew_book("dve", 1024 * 1.04 + 360)

                    post.setdefault(last_u, []).append(norm_a)
                    post.setdefault(last_u + 4, []).append(norm_b)

                # chunk tail: 8 W_O units, deferred into the next chunk
                def wo_unit(c, qs, half):
                    def go():
                        zv = zh_cs[c][:].rearrange("p (q j m) -> p q j m",
                                                   q=4, j=NP)
                        po = mspool.tile([128, 512], f32, tag="ms", name="po")
                        for j in range(NP):
                            nc.tensor.matmul(
                                po[:, 0:384], zv[:, qs, j, :],
                                wo_sb[:, j * 768 + half * 384:
                                      j * 768 + (half + 1) * 384],
                                start=(j == 0), stop=(j == NP - 1))
                        osb = opool.tile([128, 384], f32, tag="osb", name="osb")
                        eng = ew_pick(384 * 0.833, 384 * 1.04)
                        if eng == "act":
                            nc.scalar.copy(osb[:], po[:, 0:384])
                        else:
                            nc.vector.tensor_copy(osb[:], po[:, 0:384])
                        row = c * 512 + qs * 128
                        hsl = slice(half * 384, (half + 1) * 384)
                        q_e = nc.sync if (qs + half) % 2 == 0 else nc.gpsimd
                        q_e.dma_start(out[row:row + 128, hsl], osb[:])
                    return go

                base = len(units) - 1
                for k8 in range(8):
                    post.setdefault(base + 7 + k8, []).append(
                        wo_unit(c, k8 // 2, k8 % 2))

            # filler: projections for chunks 1..3 (chunk 0 done in prologue)
            def filler_units():
                for c in range(1, QC):
                    yield from proj_units(c, mspool)

            filler = iter(filler_units())
            # schedule of per-(c) unit counts to pace projections: proj for
            # chunk c+1 must complete during chunk c's units.
            NU = len(units)
            pending_post = []
            fill_budget = {}
            # chunk unit ranges
            ranges = []
            u0 = 0
            for c in range(QC):
                n = (4 * c + 4) * NP
                ranges.append((u0, u0 + n))
                u0 += n

            fired = [0]

            def fire_slot(u):
                todo = post.pop(u, None)
                if todo:
                    pending_post.extend(todo)
                if pending_post:
                    pending_post.pop(0)()
                    return
                f = next(filler, None)
                if f is not None:
                    f()

            PIPE1, PIPE3 = 1, 3
            for u in range(NU + PIPE3):
                if u < NU:
                    units[u]["scores"]()
                if 0 <= u - PIPE1 < NU:
                    units[u - PIPE1]["exp"]()
                if 0 <= u - PIPE3 < NU:
                    units[u - PIPE3]["pv"]()
                fire_slot(u - PIPE3)
            # drain remaining slots
            for u in range(NU, NU + 32):
                todo = post.pop(u - PIPE3, None)
                if todo:
                    pending_post.extend(todo)
            while pending_post:
                pending_post.pop(0)()
            for f in filler:
                f()
            assert not post, f"unfired post slots: {list(post)}"

    _split_drain_waits(nc, mybir)
    return nc


_nc_cache = None


def _prep_core_inputs(x, W_Q, W_K, W_V, W_O, b_Q, b_K, core):
    import ml_dtypes
    bf16 = ml_dtypes.bfloat16
    b, g = core // G, core % G
    hs = slice(g * HPG, (g + 1) * HPG)

    xb = np.asarray(x[b], np.float32)               # [2048, 768]
    xT = np.ascontiguousarray(xb.T).astype(bf16)

    def wqk(W):
        # [p, a, (j hh d)] <- W[2j+hh, a*128+p, d]
        w = np.asarray(W, np.float32)[hs]           # [6, 768, 64]
        return np.ascontiguousarray(
            w.reshape(3, 2, 6, 128, 64).transpose(3, 2, 0, 1, 4)
            .reshape(128, -1)).astype(bf16)

    def wvp(W):
        # [p, a, (h d)] <- W[h, a*128+p, d]
        w = np.asarray(W, np.float32)[hs]
        return np.ascontiguousarray(
            w.reshape(6, 6, 128, 64).transpose(2, 1, 0, 3)
            .reshape(128, -1)).astype(bf16)

    # wo: [p=(hh*64+d), (j dcol)] <- W_O[2j+hh, d, dcol]
    wo = np.asarray(W_O, np.float32)[hs]            # [6, 64, 768]
    wod = np.ascontiguousarray(
        wo.reshape(3, 2, 64, 768).transpose(1, 2, 0, 3)
        .reshape(128, -1)).astype(bf16)

    def bp(bias):
        bb = np.asarray(bias, np.float32)[hs]       # [6, 64]
        return np.ascontiguousarray(
            bb.reshape(3, 2, 64).transpose(1, 2, 0).reshape(128, 3))

    p = np.arange(128)[:, None]
    q = np.arange(512)[None, :]
    m1 = np.where((q < 128) & (q < p), MASKNEG, CE).astype(np.float32)
    mA = np.concatenate([m1, m1], 1)

    return {
        "xT": xT, "wq": wqk(W_Q), "wk": wqk(W_K), "wv": wvp(W_V),
        "wo": wod, "bqd": bp(b_Q), "bkd": bp(b_K), "mskd": mA,
        "ones2_d": np.ones((128, 128), np.float32),
        "ones1_d": np.ones((1, 128), bf16),
    }


def kernel(normalized_resid_pre, W_Q, W_K, W_V, W_O, b_Q, b_K, b_V, b_O):
    from concourse.bass_utils import run_bass_kernel_spmd

    global _nc_cache
    if _nc_cache is None:
        _nc_cache = build_program()
    nc = _nc_cache

    x = np.asarray(normalized_resid_pre, np.float32)
    in_maps = [_prep_core_inputs(x, W_Q, W_K, W_V, W_O, b_Q, b_K, c)
               for c in range(N_CORES)]

    res = run_bass_kernel_spmd(nc, in_maps, core_ids=list(range(N_CORES)))
    out = np.zeros((B, S, D), np.float32)
    for c in range(N_CORES):
        out[c // G] += np.asarray(res.results[c]["out"], np.float32)
    # bias folds: b_V rides through softmax rows (sum to 1) into W_O
    out += np.asarray(b_O, np.float32)
    out += np.einsum("nh,nhd->d", np.asarray(b_V, np.float32),
                     np.asarray(W_O, np.float32))
    return out
